# revision 13
# baseline (speedup 1.0000x reference)
"""Trainium2 Bass kernel for batched channel attention — all-fp8 DoubleRow.

Reference computation (per batch b; B=8, A=2048 tokens, D=1024 dims):
    q = x @ Wq.T ; k = x @ Wk.T ; v = x @ Wv.T          # (A, D)
    q,k,v -> (D, A); q,k L2-normalized over the token axis
    attn = softmax((qn @ kn.T) * temperature, axis=-1)   # (D, D)
    out  = attn @ v_da                                   # (D, A)
    y    = out.T @ Wo.T                                  # (A, D)

Numerics: all six GEMMs run in fp8 e4m3 with DoubleRow perf mode.
The normalize+softmax path is fp8-insensitive (errors divide by the
2048-token normalization).  The value path uses a mean-centered
decomposition: since the softmax here is near-uniform, P = exp(Sn) =
1 + dP with |dP| ~ 0.02, so
    attn @ v = 1 (x) colsum_v + dP @ v
    y = colsum_v (x) (invden @ Wo.T) + (invden*dPv).T @ Wo.T
The rank-1 dominant term is carried in bf16 (exactly, via K=1 matmuls
accumulated into the same PSUM); only the small delta runs in fp8,
scaled up into e4m3's precision sweet spot.  CPU-simulated rel err
vs the fp32 reference: ~6e-3 (gate: 2e-2).

Layouts: DoubleRow packs two K-tiles per instruction; every fp8
operand is stored [128 part, 2, free] with global contraction row
j*128+p in [:, j, :].  The softmax is computed TRANSPOSED (partition
= k-feature e) so no 128x128 PE transposes are needed anywhere, and
y is produced transposed (f, a) so the out@Wo GEMM amortizes each
weight load over 4 chunk matmuls (host transposes it back).
"""

import numpy as np

B, A, D = 8, 2048, 1024
P = 128
NPAIR = D // 256     # 4 k-tile pairs per 1024-dim contraction
A_T = A // P         # 16 token tiles
NCH = 512

ALPHA = 16.0         # fp8 weight scale
DP_SCALE = 32.0      # deltaP = (exp(Sn)-1)*DP_SCALE
D2_SCALE = 512.0     # delta2 eviction scale

_CACHE = {}


def _ensure_path():
    import importlib.util
    import sys
    if importlib.util.find_spec("concourse") is None:
        sys.path.insert(0, "/opt/trn_rl_repo")


def build_bass():
    _ensure_path()
    import concourse.bacc as bacc
    import concourse.mybir as mybir
    import concourse.tile as tile

    dt = mybir.dt
    BF = dt.bfloat16
    F8 = dt.float8e4
    F32 = dt.float32
    AF = mybir.ActivationFunctionType
    MULT = mybir.AluOpType.mult
    SUB = mybir.AluOpType.subtract
    ADD = mybir.AluOpType.add
    DR = mybir.MatmulPerfMode.DoubleRow

    nc = bacc.Bacc()

    # fp8 pair layouts: row pr*128+p, col j*W + c
    x8_d = nc.declare_dram_parameter("x8", [NPAIR * P, 2 * A], F8, isOutput=False)
    xb_d = nc.declare_dram_parameter("xb", [D, A], BF, isOutput=False)
    wq_d = nc.declare_dram_parameter("wq8", [NPAIR * P, 2 * D], F8, isOutput=False)
    wk_d = nc.declare_dram_parameter("wk8", [NPAIR * P, 2 * D], F8, isOutput=False)
    wv_d = nc.declare_dram_parameter("wv8", [NPAIR * P, 2 * D], F8, isOutput=False)
    wo_d = nc.declare_dram_parameter("wo8", [NPAIR * P, 2 * D], F8, isOutput=False)
    wob_d = nc.declare_dram_parameter("wob", [D, D], BF, isOutput=False)
    wvc_d = nc.declare_dram_parameter("wvc", [P, D // P], BF, isOutput=False)
    tp_d = nc.declare_dram_parameter("temp", [1, 1], F32, isOutput=False)
    y_d = nc.declare_dram_parameter("y", [D, A], BF, isOutput=True)  # yT (f, a)

    with tile.TileContext(nc) as tc:
        # ---- pools, stack order = reverse release order ----
        consts = tc.alloc_tile_pool(name="consts", bufs=1)
        misc = tc.alloc_tile_pool(name="misc", bufs=1)
        wo8_pool = tc.alloc_tile_pool(name="wo8p", bufs=NPAIR)
        d2_pool = tc.alloc_tile_pool(name="d2p", bufs=NPAIR)
        v8_pool = tc.alloc_tile_pool(name="v8p", bufs=NPAIR)
        dp_pool = tc.alloc_tile_pool(name="dpp", bufs=NPAIR)
        x8_pool = tc.alloc_tile_pool(name="x8p", bufs=NPAIR, side="right")
        xb_pool = tc.alloc_tile_pool(name="xbp", bufs=8, side="right")
        wv8_pool = tc.alloc_tile_pool(name="wv8p", bufs=NPAIR)
        wob_pool = tc.alloc_tile_pool(name="wobp", bufs=8)
        qk_pool = tc.alloc_tile_pool(name="qkp", bufs=A_T // 2)
        bcast_pool = tc.alloc_tile_pool(name="bcp", bufs=1)
        bcast_sb = bcast_pool.tile([P, D], F32, tag="bcast")
        wqk_pool = tc.alloc_tile_pool(name="wqkp", bufs=NPAIR)

        # constants
        one11 = consts.tile([1, 1], F32, tag="one11")
        nc.vector.memset(one11[:], 1.0)
        ones_row = consts.tile([1, P], F32, tag="ones_row")
        nc.vector.memset(ones_row[:], 1.0)
        ones8_t = consts.tile([P, 2, 16], F8, tag="ones8")
        nc.vector.memset(ones8_t[:], 1.0)
        ones8 = ones8_t[:, :, 0:1]
        t_sb = consts.tile([1, 1], F32, tag="t_sb")
        nc.sync.dma_start(t_sb[:], tp_d[:])
        invnk_col = consts.tile([P, D // P], F32, tag="invnk_col")
        invden_col = consts.tile([P, D // P], F32, tag="invden_col")
        invden_bf = consts.tile([P, D // P], BF, tag="invden_bf")
        sc2_col = consts.tile([P, D // P], F32, tag="sc2_col")
        wvc = consts.tile([P, D // P], BF, tag="wvc")
        nc.sync.dma_start(wvc[:], wvc_d[:])
        # warmup operand: dummy matmuls bridge the DMA-bound startup so the
        # PE clock ramps and never demotes (idle >3us drops it to half rate)
        wtile = consts.tile([P, NCH], BF, tag="wt")
        nc.vector.memset(wtile[:], 0.25)


        # input tiles.  DMA issue order is load-bearing: phase-1 operands
        # (x8 + wq8/wk8) go first, interleaved per pair so the first
        # accumulation chain can start ~3us in; later-phase tensors queue
        # behind them.
        def load_w8(pool, dram, nm, eng=None, engs=None):
            ws = []
            for i in range(NPAIR):
                t = pool.tile([P, 2, D], F8, tag=nm, name=f"{nm}{i}")
                e = engs[i % 2] if engs else eng
                if e is not None:
                    e.dma_start(t[:], dram[i * P:(i + 1) * P, :])
                ws.append(t)
            return ws

        x8s = [x8_pool.tile([P, 2, A], F8, tag="x8", name=f"x8_{i}")
               for i in range(NPAIR)]
        warm_ps = tc.alloc_tile_pool(name="warm_ps", bufs=1, space="PSUM")
        wacc = warm_ps.tile([P, NCH], F32, tag="wacc", name="wacc")
        for i in range(40):
            nc.tensor.matmul(wacc[:], wtile[:, 0:P], wtile[:],
                             start=(i == 0), stop=(i == 39))
        warm_ps.release()

        wq8s = load_w8(wqk_pool, wq_d, "wq")
        wk8s = load_w8(wqk_pool, wk_d, "wk")
        for pr in range(NPAIR):
            eng = nc.gpsimd if pr % 2 == 0 else nc.scalar
            r = slice(pr * P, (pr + 1) * P)
            eng.dma_start(x8s[pr][:], x8_d[r, :])
            eng.dma_start(wq8s[pr][:], wq_d[r, :])
            eng.dma_start(wk8s[pr][:], wk_d[r, :])
        xbs = []
        for i in range(8):
            t = xb_pool.tile([P, A], BF, tag="xb", name=f"xb_{i}")
            xbs.append(t)
            nc.sync.dma_start(t[:], xb_d[i * P:(i + 1) * P, :])
        wv8s = load_w8(wv8_pool, wv_d, "wv", engs=[nc.gpsimd, nc.scalar])
        wobs = []
        for i in range(8):
            t = wob_pool.tile([P, D], BF, tag="wob", name=f"wob{i}")
            nc.scalar.dma_start(t[:], wob_d[i * P:(i + 1) * P, :])
            wobs.append(t)
        wo8s = load_w8(wo8_pool, wo_d, "wo", nc.sync)

        q8s = [qk_pool.tile([P, 2, D], F8, tag="q", name=f"q{i}")
               for i in range(A_T // 2)]
        k8s = [qk_pool.tile([P, 2, D], F8, tag="k", name=f"k{i}")
               for i in range(A_T // 2)]

        # ---------- phase 1: q/k projections + token-axis sumsq ----------
        sq_pool = tc.alloc_tile_pool(name="sqp", bufs=3, side="right")
        # PSUM: nrm first (lives through phase 3), then qk ring (6 banks)
        nrm_ps = tc.alloc_tile_pool(name="nrm_ps", bufs=1, space="PSUM",
                                    side="right")
        qk_ps = tc.alloc_tile_pool(name="qk_ps", bufs=2, space="PSUM")

        def proj_pass(ws, dst, interlude=None):
            # sq ring of 3; each pair's norm matmuls are emitted two tiles
            # late so the PE never waits on the ACT squares
            ns = nrm_ps.tile([1, D], F32, tag="nrm", name="ns")
            pending = []

            def emit_norm(sq_t, first, last):
                for c in range(D // NCH):
                    nc.tensor.matmul(
                        ns[:, c * NCH:(c + 1) * NCH],
                        ones8,
                        sq_t[:, :, c * NCH:(c + 1) * NCH],
                        start=first,
                        stop=last,
                        perf_mode=DR,
                    )

            sq = None
            for ai in range(A_T):
                j = ai % 2
                acc = qk_ps.tile([P, D], F32, tag=f"qk{j}", name=f"acc{j}",
                                 bufs=(2 if j == 0 else 1))
                for pr in range(NPAIR):
                    lhs = x8s[pr][:, :, ai * P:(ai + 1) * P]
                    for c in range(D // NCH):
                        nc.tensor.matmul(
                            acc[:, c * NCH:(c + 1) * NCH],
                            lhs,
                            ws[pr][:, :, c * NCH:(c + 1) * NCH],
                            start=(pr == 0),
                            stop=(pr == NPAIR - 1),
                            perf_mode=DR,
                        )
                nc.vector.tensor_copy(dst[ai // 2][:, j, :], acc[:])
                if j == 0:
                    sq = sq_pool.tile([P, 2, D], F8, tag="sq", name="sq")
                nc.scalar.activation(sq[:, j, :], acc[:], AF.Square,
                                     scale=1.0 / ALPHA)
                if j == 1:
                    pending.append(sq)
                if len(pending) > 1:
                    emit_norm(pending.pop(0), first=(ai == 3), last=False)
                if interlude is not None and ai == 1:
                    interlude()
            emit_norm(pending.pop(0), first=False, last=True)
            return ns

        nq_row = misc.tile([1, D], F32, tag="nq_row")
        nk_row = misc.tile([1, D], F32, tag="nk_row")
        r1_row = misc.tile([1, D], F32, tag="r1_row")
        t_inv = misc.tile([1, 1], F32, tag="t_inv")

        ns_q = proj_pass(wq8s, q8s)
        nc.scalar.activation(nq_row[:], ns_q[:], AF.Sqrt)

        def i_qchain():
            # r1_row = nq * ALPHA^2 / temp, then partition-broadcast via a
            # K=1 matmul and reciprocal -> bcast_sb, all under the k-pass
            nc.vector.reciprocal(t_inv[:], t_sb[:])
            nc.vector.tensor_scalar(
                out=r1_row[:], in0=nq_row[:],
                scalar1=t_inv[0:1, 0:1], scalar2=ALPHA * ALPHA,
                op0=MULT, op1=MULT,
            )
            bc_ps = nrm_ps.tile([P, D], F32, tag="nrm", name="bc_ps")
            for c in range(D // NCH):
                nc.tensor.matmul(
                    bc_ps[:, c * NCH:(c + 1) * NCH],
                    ones_row[:],
                    r1_row[0:1, c * NCH:(c + 1) * NCH],
                )
            nc.vector.reciprocal(bcast_sb[:], bc_ps[:])

        ns_k = proj_pass(wk8s, k8s, interlude=i_qchain)
        nc.scalar.activation(nk_row[:], ns_k[:], AF.Sqrt)

        sq_pool.release()
        wqk_pool.release()
        qk_ps.release()

        # ---------- phase 2: transposed scores + softmax deltas ----------
        # PSUM: smallcol (1 bank) + s ring (2x2) + nrm (2) = 7
        smallcol_ps = tc.alloc_tile_pool(name="smc_ps", bufs=1, space="PSUM")
        s_ps_pool = tc.alloc_tile_pool(name="s_ps", bufs=2, space="PSUM")
        scr_pool = tc.alloc_tile_pool(name="scrp", bufs=2)
        exp_pool = tc.alloc_tile_pool(name="expp", bufs=2)

        dp8s = [dp_pool.tile([P, 2, D], F8, tag="dp", name=f"dp{i}")
                for i in range(NPAIR)]

        def s_mms(ej):
            s_ps = s_ps_pool.tile([P, D], F32, tag="s", name="s_ps")
            for pr in range(A_T // 2):
                lhs = k8s[pr][:, :, ej * P:(ej + 1) * P]
                for c in range(D // NCH):
                    nc.tensor.matmul(
                        s_ps[:, c * NCH:(c + 1) * NCH],
                        lhs,
                        q8s[pr][:, :, c * NCH:(c + 1) * NCH],
                        start=(pr == 0),
                        stop=(pr == A_T // 2 - 1),
                        perf_mode=DR,
                    )
            return s_ps

        def s_evict(ej, s_ps):
            s_scr = scr_pool.tile([P, D], F32, tag="s_scr", name="s_scr")
            nc.vector.tensor_tensor(s_scr[:], s_ps[:], bcast_sb[:], MULT)
            e_sb = exp_pool.tile([P, D], F32, tag="exp", name="e_sb")
            nc.scalar.activation(e_sb[:], s_scr[:], AF.Exp,
                                 scale=invnk_col[:, ej:ej + 1])
            nc.vector.tensor_scalar(
                out=dp8s[ej // 2][:, ej % 2, :], in0=e_sb[:],
                scalar1=1.0, scalar2=DP_SCALE, op0=SUB, op1=MULT,
            )

        # invnk column-ization: 8 tiny transposes + reciprocal
        nkc_ps = smallcol_ps.tile([P, D // P], F32, tag="smc", name="nkc_ps")
        for j in range(D // P):
            nc.tensor.transpose(nkc_ps[:, j:j + 1],
                                nk_row[0:1, j * P:(j + 1) * P], one11[:])
        nc.vector.reciprocal(invnk_col[:], nkc_ps[:])

        for ej in range(D // P):
            s_evict(ej, s_mms(ej))

        s_ps_pool.release()

        # ---------- phase 3: v projection (+ denom / colsum / iw chains) --
        v_ps_pool = tc.alloc_tile_pool(name="v_ps", bufs=2, space="PSUM")
        v8s = [v8_pool.tile([P, 2, A], F8, tag="v8", name=f"v8_{i}")
               for i in range(NPAIR)]
        dn_row = misc.tile([1, D], F32, tag="dn_row")
        cs_row = misc.tile([1, A], BF, tag="cs_row")
        iw_row = misc.tile([1, D], BF, tag="iw_row")

        def v_mms(dj, h):
            vp = v_ps_pool.tile([P, A // 2], F32, tag="vps", name="vp")
            for pr in range(NPAIR):
                lhs = wv8s[pr][:, :, dj * P:(dj + 1) * P]
                for c in range(2):
                    off = h * (A // 2) + c * NCH
                    nc.tensor.matmul(
                        vp[:, c * NCH:(c + 1) * NCH],
                        lhs,
                        x8s[pr][:, :, off:off + NCH],
                        start=(pr == 0),
                        stop=(pr == NPAIR - 1),
                        perf_mode=DR,
                    )
            nc.scalar.activation(
                v8s[dj // 2][:, dj % 2, h * (A // 2):(h + 1) * (A // 2)],
                vp[:], AF.Copy)

        def dn_mms():
            # denom row from quantized dP: ones8 @ dP  (+1024 after /32)
            dn_ps = nrm_ps.tile([1, D], F32, tag="nrm", name="dn_ps")
            for pr in range(NPAIR):
                for c in range(D // NCH):
                    nc.tensor.matmul(
                        dn_ps[:, c * NCH:(c + 1) * NCH],
                        ones8,
                        dp8s[pr][:, :, c * NCH:(c + 1) * NCH],
                        start=(pr == 0),
                        stop=(pr == NPAIR - 1),
                        perf_mode=DR,
                    )
            nc.vector.tensor_scalar(
                out=dn_row[:], in0=dn_ps[:],
                scalar1=1.0 / DP_SCALE, scalar2=float(D),
                op0=MULT, op1=ADD,
            )

        def dn_cols():
            dnc_ps = smallcol_ps.tile([P, D // P], F32, tag="smc",
                                      name="dnc_ps")
            for j in range(D // P):
                nc.tensor.transpose(dnc_ps[:, j:j + 1],
                                    dn_row[0:1, j * P:(j + 1) * P], one11[:])
            nc.vector.reciprocal(invden_col[:], dnc_ps[:])
            nc.vector.tensor_copy(invden_bf[:], invden_col[:])
            nc.vector.tensor_scalar(
                out=sc2_col[:], in0=invden_col[:],
                scalar1=D2_SCALE / (DP_SCALE * ALPHA), scalar2=None, op0=MULT,
            )

        def cs_mms(h):
            cs_ps = nrm_ps.tile([1, A // 2], F32, tag="nrm", name="cs_ps")
            for ft in range(8):
                lhs = wvc[:, ft:ft + 1]
                for c in range(2):
                    off = h * (A // 2) + c * NCH
                    nc.tensor.matmul(
                        cs_ps[:, c * NCH:(c + 1) * NCH],
                        lhs,
                        xbs[ft][:, off:off + NCH],
                        start=(ft == 0),
                        stop=(ft == 7),
                    )
            nc.vector.tensor_copy(
                cs_row[0:1, h * (A // 2):(h + 1) * (A // 2)], cs_ps[:])

        def iw_mms():
            iw_ps = nrm_ps.tile([1, D], F32, tag="nrm", name="iw_ps")
            for dj in range(8):
                lhs = invden_bf[:, dj:dj + 1]
                for c in range(D // NCH):
                    nc.tensor.matmul(
                        iw_ps[:, c * NCH:(c + 1) * NCH],
                        lhs,
                        wobs[dj][:, c * NCH:(c + 1) * NCH],
                        start=(dj == 0),
                        stop=(dj == 7),
                    )
            nc.vector.tensor_scalar(
                out=iw_row[:], in0=iw_ps[:],
                scalar1=D2_SCALE * ALPHA, scalar2=None, op0=MULT,
            )

        interludes = {1: dn_mms, 3: dn_cols, 5: lambda: cs_mms(0),
                      7: lambda: cs_mms(1), 9: iw_mms}
        step = 0
        for dj in range(8):
            for h in range(2):
                v_mms(dj, h)
                step += 1
                if step in interludes:
                    interludes[step]()

        v_ps_pool.release()
        smallcol_ps.release()
        nrm_ps.release()
        exp_pool.release()
        scr_pool.release()
        bcast_pool.release()
        qk_pool.release()

        # ---------- phase 4: delta2 = invden * (dP @ v) ----------
        d2_ps_pool = tc.alloc_tile_pool(name="d2_ps", bufs=2, space="PSUM")
        d2s = [d2_pool.tile([P, 2, A], F8, tag="d2", name=f"d2_{i}")
               for i in range(NPAIR)]
        for dj in range(8):
            dp_ = d2_ps_pool.tile([P, A], F32, tag="d2ps", name="dp_")
            for pr in range(NPAIR):
                lhs = dp8s[pr][:, :, dj * P:(dj + 1) * P]
                for c in range(A // NCH):
                    nc.tensor.matmul(
                        dp_[:, c * NCH:(c + 1) * NCH],
                        lhs,
                        v8s[pr][:, :, c * NCH:(c + 1) * NCH],
                        start=(pr == 0),
                        stop=(pr == NPAIR - 1),
                        perf_mode=DR,
                    )
            nc.scalar.activation(d2s[dj // 2][:, dj % 2, :], dp_[:], AF.Copy,
                                 scale=sc2_col[:, dj:dj + 1])
        d2_ps_pool.release()

        # ---------- phase 5: yT = wo8.T-ish GEMM + rank-1 ----------
        y_ps_pool = tc.alloc_tile_pool(name="y_ps", bufs=2, space="PSUM")
        y_pool = tc.alloc_tile_pool(name="yp", bufs=2)
        for fj in range(8):
            yp = y_ps_pool.tile([P, A], F32, tag="yps", name="yp_t")
            for pr in range(NPAIR):
                lhs = wo8s[pr][:, :, fj * P:(fj + 1) * P]
                for c in range(A // NCH):
                    nc.tensor.matmul(
                        yp[:, c * NCH:(c + 1) * NCH],
                        lhs,
                        d2s[pr][:, :, c * NCH:(c + 1) * NCH],
                        start=(pr == 0),
                        stop=False,
                        perf_mode=DR,
                    )
            for c in range(A // NCH):
                nc.tensor.matmul(
                    yp[:, c * NCH:(c + 1) * NCH],
                    iw_row[0:1, fj * P:(fj + 1) * P],
                    cs_row[0:1, c * NCH:(c + 1) * NCH],
                    start=False,
                    stop=True,
                )
            y_sb = y_pool.tile([P, A], BF, tag="y", name="y_sb")
            for h in range(2):
                sl = slice(h * (A // 2), (h + 1) * (A // 2))
                if fj % 2 == 0:
                    nc.vector.tensor_scalar(
                        out=y_sb[:, sl], in0=yp[:, sl],
                        scalar1=1.0 / (D2_SCALE * ALPHA), scalar2=None,
                        op0=MULT,
                    )
                else:
                    nc.scalar.activation(y_sb[:, sl], yp[:, sl], AF.Copy,
                                         scale=1.0 / (D2_SCALE * ALPHA))
                nc.sync.dma_start(
                    y_d[fj * P:(fj + 1) * P, sl], y_sb[:, sl])

        y_pool.release()
        y_ps_pool.release()
        wob_pool.release()
        wv8_pool.release()
        xb_pool.release()
        x8_pool.release()
        dp_pool.release()
        v8_pool.release()
        d2_pool.release()
        wo8_pool.release()
        misc.release()
        consts.release()

    nc.compile()
    return nc


def _pair_layout(mT):
    """[K, M] -> DoubleRow pair layout [K/256*128, 2*M] (row pr*128+p)."""
    K, M = mT.shape
    return np.ascontiguousarray(
        mT.reshape(K // 256, 2, P, M).transpose(0, 2, 1, 3).reshape(K // 2, 2 * M))


def _host_inputs(x, Wq, Wk, Wv, Wo, temperature):
    import ml_dtypes
    f8 = ml_dtypes.float8_e4m3
    bf16 = ml_dtypes.bfloat16

    def to8(a):
        return np.clip(a, -239.0, 239.0).astype(f8)

    wq8 = _pair_layout(to8(ALPHA * np.asarray(Wq).T))
    wk8 = _pair_layout(to8(ALPHA * np.asarray(Wk).T))
    wv8 = _pair_layout(to8(ALPHA * np.asarray(Wv).T))
    wo8 = _pair_layout(to8(ALPHA * np.asarray(Wo).T))
    wob = np.ascontiguousarray(np.asarray(Wo).T).astype(bf16)
    wvc = np.ascontiguousarray(
        np.asarray(Wv).sum(0).reshape(D // P, P).T).astype(bf16)
    in_maps = []
    for b in range(B):
        xT = np.ascontiguousarray(np.asarray(x[b]).T)
        in_maps.append({
            "x8": _pair_layout(to8(xT)),
            "xb": xT.astype(bf16),
            "wq8": wq8, "wk8": wk8, "wv8": wv8, "wo8": wo8,
            "wob": wob, "wvc": wvc,
            "temp": np.asarray(temperature[b]).reshape(1, 1).astype(np.float32),
        })
    return in_maps


def run(x, Wq, Wk, Wv, Wo, temperature, trace=False, tmpdir=None):
    _ensure_path()
    from concourse.bass_utils import run_bass_kernel_spmd

    if "nc" not in _CACHE:
        _CACHE["nc"] = build_bass()
    nc = _CACHE["nc"]
    in_maps = _host_inputs(x, Wq, Wk, Wv, Wo, temperature)
    res = run_bass_kernel_spmd(
        nc, in_maps, core_ids=list(range(B)), trace=trace, tmpdir=tmpdir
    )
    out = np.stack([
        np.asarray(res.results[b]["y"]).astype(np.float32).T for b in range(B)
    ])
    return out, res


def kernel(x, Wq, Wk, Wv, Wo, temperature):
    out, _ = run(x, Wq, Wk, Wv, Wo, temperature, trace=False)
    return out


# revision 14
# speedup vs baseline: 1.1589x; 1.1589x over previous
"""Trainium2 Bass kernel for batched channel attention — all-fp8 DoubleRow.

Reference computation (per batch b; B=8, A=2048 tokens, D=1024 dims):
    q = x @ Wq.T ; k = x @ Wk.T ; v = x @ Wv.T          # (A, D)
    q,k,v -> (D, A); q,k L2-normalized over the token axis
    attn = softmax((qn @ kn.T) * temperature, axis=-1)   # (D, D)
    out  = attn @ v_da                                   # (D, A)
    y    = out.T @ Wo.T                                  # (A, D)

Numerics: all six GEMMs run in fp8 e4m3 with DoubleRow perf mode.
The normalize+softmax path is fp8-insensitive (errors divide by the
2048-token normalization).  The value path uses a mean-centered
decomposition: since the softmax here is near-uniform, P = exp(Sn) =
1 + dP with |dP| ~ 0.02, so
    attn @ v = 1 (x) colsum_v + dP @ v
    y = colsum_v (x) (invden @ Wo.T) + (invden*dPv).T @ Wo.T
The rank-1 dominant term is carried in bf16 (exactly, via K=1 matmuls
accumulated into the same PSUM); only the small delta runs in fp8,
scaled up into e4m3's precision sweet spot.  CPU-simulated rel err
vs the fp32 reference: ~6e-3 (gate: 2e-2).

Layouts: DoubleRow packs two K-tiles per instruction; every fp8
operand is stored [128 part, 2, free] with global contraction row
j*128+p in [:, j, :].  The softmax is computed TRANSPOSED (partition
= k-feature e) so no 128x128 PE transposes are needed anywhere, and
y is produced transposed (f, a) so the out@Wo GEMM amortizes each
weight load over 4 chunk matmuls (host transposes it back).
"""

import numpy as np

B, A, D = 8, 2048, 1024
P = 128
NPAIR = D // 256     # 4 k-tile pairs per 1024-dim contraction
A_T = A // P         # 16 token tiles
NCH = 512

ALPHA = 16.0         # fp8 weight scale
DP_SCALE = 32.0      # deltaP = (exp(Sn)-1)*DP_SCALE
D2_SCALE = 512.0     # delta2 eviction scale

_CACHE = {}


def _ensure_path():
    import importlib.util
    import sys
    if importlib.util.find_spec("concourse") is None:
        sys.path.insert(0, "/opt/trn_rl_repo")


def build_bass():
    _ensure_path()
    import concourse.bacc as bacc
    import concourse.mybir as mybir
    import concourse.tile as tile

    dt = mybir.dt
    BF = dt.bfloat16
    F8 = dt.float8e4
    F32 = dt.float32
    AF = mybir.ActivationFunctionType
    MULT = mybir.AluOpType.mult
    SUB = mybir.AluOpType.subtract
    ADD = mybir.AluOpType.add
    DR = mybir.MatmulPerfMode.DoubleRow

    nc = bacc.Bacc()

    # fp8 pair layouts: row pr*128+p, col j*W + c
    x8_d = nc.declare_dram_parameter("x8", [NPAIR * P, 2 * A], F8, isOutput=False)
    xb_d = nc.declare_dram_parameter("xb", [D, A], BF, isOutput=False)
    wq_d = nc.declare_dram_parameter("wq8", [NPAIR * P, 2 * D], F8, isOutput=False)
    wk_d = nc.declare_dram_parameter("wk8", [NPAIR * P, 2 * D], F8, isOutput=False)
    wv_d = nc.declare_dram_parameter("wv8", [NPAIR * P, 2 * D], F8, isOutput=False)
    wo_d = nc.declare_dram_parameter("wo8", [NPAIR * P, 2 * D], F8, isOutput=False)
    wob_d = nc.declare_dram_parameter("wob", [D, D], BF, isOutput=False)
    wvc_d = nc.declare_dram_parameter("wvc", [P, D // P], BF, isOutput=False)
    tp_d = nc.declare_dram_parameter("temp", [1, 1], F32, isOutput=False)
    y_d = nc.declare_dram_parameter("y", [D, A], BF, isOutput=True)  # yT (f, a)

    with tile.TileContext(nc) as tc:
        # ---- pools, stack order = reverse release order ----
        consts = tc.alloc_tile_pool(name="consts", bufs=1)
        misc = tc.alloc_tile_pool(name="misc", bufs=1)
        wo8_pool = tc.alloc_tile_pool(name="wo8p", bufs=NPAIR)
        d2_pool = tc.alloc_tile_pool(name="d2p", bufs=NPAIR)
        v8_pool = tc.alloc_tile_pool(name="v8p", bufs=NPAIR)
        dp_pool = tc.alloc_tile_pool(name="dpp", bufs=NPAIR)
        x8_pool = tc.alloc_tile_pool(name="x8p", bufs=NPAIR, side="right")
        xb_pool = tc.alloc_tile_pool(name="xbp", bufs=8, side="right")
        wv8_pool = tc.alloc_tile_pool(name="wv8p", bufs=NPAIR)
        wob_pool = tc.alloc_tile_pool(name="wobp", bufs=8)
        qk_pool = tc.alloc_tile_pool(name="qkp", bufs=A_T // 2)
        bcast_pool = tc.alloc_tile_pool(name="bcp", bufs=1)
        bcast_sb = bcast_pool.tile([P, D], F32, tag="bcast")
        wqk_pool = tc.alloc_tile_pool(name="wqkp", bufs=NPAIR)

        # constants
        one11 = consts.tile([1, 1], F32, tag="one11")
        nc.vector.memset(one11[:], 1.0)
        ones_row = consts.tile([1, P], F32, tag="ones_row")
        nc.vector.memset(ones_row[:], 1.0)
        ones8_t = consts.tile([P, 2, 16], F8, tag="ones8")
        nc.vector.memset(ones8_t[:], 1.0)
        ones8 = ones8_t[:, :, 0:1]
        t_sb = consts.tile([1, 1], F32, tag="t_sb")
        nc.sync.dma_start(t_sb[:], tp_d[:])
        invnk_col = consts.tile([P, D // P], F32, tag="invnk_col")
        invden_col = consts.tile([P, D // P], F32, tag="invden_col")
        invden_bf = consts.tile([P, D // P], BF, tag="invden_bf")
        sc2_col = consts.tile([P, D // P], F32, tag="sc2_col")
        wvc = consts.tile([P, D // P], BF, tag="wvc")
        nc.sync.dma_start(wvc[:], wvc_d[:])




        # input tiles.  DMA issue order is load-bearing: phase-1 operands
        # (x8 + wq8/wk8) go first, interleaved per pair so the first
        # accumulation chain can start ~3us in; later-phase tensors queue
        # behind them.
        def load_w8(pool, dram, nm, eng=None, engs=None):
            ws = []
            for i in range(NPAIR):
                t = pool.tile([P, 2, D], F8, tag=nm, name=f"{nm}{i}")
                e = engs[i % 2] if engs else eng
                if e is not None:
                    e.dma_start(t[:], dram[i * P:(i + 1) * P, :])
                ws.append(t)
            return ws

        x8s = [x8_pool.tile([P, 2, A], F8, tag="x8", name=f"x8_{i}")
               for i in range(NPAIR)]
        wq8s = load_w8(wqk_pool, wq_d, "wq")
        wk8s = load_w8(wqk_pool, wk_d, "wk")
        for pr in range(NPAIR):
            eng = nc.gpsimd if pr % 2 == 0 else nc.scalar
            r = slice(pr * P, (pr + 1) * P)
            eng.dma_start(x8s[pr][:], x8_d[r, :])
            eng.dma_start(wq8s[pr][:], wq_d[r, :])
            eng.dma_start(wk8s[pr][:], wk_d[r, :])
        xbs = []
        for i in range(8):
            t = xb_pool.tile([P, A], BF, tag="xb", name=f"xb_{i}")
            xbs.append(t)
            nc.sync.dma_start(t[:], xb_d[i * P:(i + 1) * P, :])
        wv8s = load_w8(wv8_pool, wv_d, "wv", engs=[nc.gpsimd, nc.scalar])
        wobs = []
        for i in range(8):
            t = wob_pool.tile([P, D], BF, tag="wob", name=f"wob{i}")
            nc.scalar.dma_start(t[:], wob_d[i * P:(i + 1) * P, :])
            wobs.append(t)
        wo8s = load_w8(wo8_pool, wo_d, "wo", nc.sync)

        q8s = [qk_pool.tile([P, 2, D], F8, tag="q", name=f"q{i}")
               for i in range(A_T // 2)]
        k8s = [qk_pool.tile([P, 2, D], F8, tag="k", name=f"k{i}")
               for i in range(A_T // 2)]

        # ---------- phase 1: q/k projections + token-axis sumsq ----------
        sq_pool = tc.alloc_tile_pool(name="sqp", bufs=3, side="right")
        # PSUM: nrm first (lives through phase 3), then qk ring (6 banks)
        nrm_ps = tc.alloc_tile_pool(name="nrm_ps", bufs=1, space="PSUM",
                                    side="right")
        qk_ps = tc.alloc_tile_pool(name="qk_ps", bufs=2, space="PSUM")

        def proj_pass(ws, dst, interlude=None):
            # sq ring of 3; each pair's norm matmuls are emitted two tiles
            # late so the PE never waits on the ACT squares
            ns = nrm_ps.tile([1, D], F32, tag="nrm", name="ns")
            pending = []

            def emit_norm(sq_t, first, last):
                for c in range(D // NCH):
                    nc.tensor.matmul(
                        ns[:, c * NCH:(c + 1) * NCH],
                        ones8,
                        sq_t[:, :, c * NCH:(c + 1) * NCH],
                        start=first,
                        stop=last,
                        perf_mode=DR,
                    )

            sq = None
            for ai in range(A_T):
                j = ai % 2
                acc = qk_ps.tile([P, D], F32, tag=f"qk{j}", name=f"acc{j}",
                                 bufs=(2 if j == 0 else 1))
                for pr in range(NPAIR):
                    lhs = x8s[pr][:, :, ai * P:(ai + 1) * P]
                    for c in range(D // NCH):
                        nc.tensor.matmul(
                            acc[:, c * NCH:(c + 1) * NCH],
                            lhs,
                            ws[pr][:, :, c * NCH:(c + 1) * NCH],
                            start=(pr == 0),
                            stop=(pr == NPAIR - 1),
                            perf_mode=DR,
                        )
                nc.vector.tensor_copy(dst[ai // 2][:, j, :], acc[:])
                if j == 0:
                    sq = sq_pool.tile([P, 2, D], F8, tag="sq", name="sq")
                nc.scalar.activation(sq[:, j, :], acc[:], AF.Square,
                                     scale=1.0 / ALPHA)
                if j == 1:
                    pending.append(sq)
                if len(pending) > 1:
                    emit_norm(pending.pop(0), first=(ai == 3), last=False)
                if interlude is not None and ai == 1:
                    interlude()
            emit_norm(pending.pop(0), first=False, last=True)
            return ns

        nq_row = misc.tile([1, D], F32, tag="nq_row")
        nk_row = misc.tile([1, D], F32, tag="nk_row")
        r1_row = misc.tile([1, D], F32, tag="r1_row")
        t_inv = misc.tile([1, 1], F32, tag="t_inv")

        ns_q = proj_pass(wq8s, q8s)
        nc.scalar.activation(nq_row[:], ns_q[:], AF.Sqrt)

        def i_qchain():
            # r1_row = nq * ALPHA^2 / temp, then partition-broadcast via a
            # K=1 matmul and reciprocal -> bcast_sb, all under the k-pass
            nc.vector.reciprocal(t_inv[:], t_sb[:])
            nc.vector.tensor_scalar(
                out=r1_row[:], in0=nq_row[:],
                scalar1=t_inv[0:1, 0:1], scalar2=ALPHA * ALPHA,
                op0=MULT, op1=MULT,
            )
            bc_ps = nrm_ps.tile([P, D], F32, tag="nrm", name="bc_ps")
            for c in range(D // NCH):
                nc.tensor.matmul(
                    bc_ps[:, c * NCH:(c + 1) * NCH],
                    ones_row[:],
                    r1_row[0:1, c * NCH:(c + 1) * NCH],
                )
            nc.vector.reciprocal(bcast_sb[:], bc_ps[:])

        ns_k = proj_pass(wk8s, k8s, interlude=i_qchain)
        nc.scalar.activation(nk_row[:], ns_k[:], AF.Sqrt)

        sq_pool.release()
        wqk_pool.release()
        qk_ps.release()

        # ---------- phase 2: transposed scores + softmax deltas ----------
        # PSUM: smallcol (1 bank) + s ring (2x2) + nrm (2) = 7
        smallcol_ps = tc.alloc_tile_pool(name="smc_ps", bufs=1, space="PSUM")
        s_ps_pool = tc.alloc_tile_pool(name="s_ps", bufs=2, space="PSUM")
        scr_pool = tc.alloc_tile_pool(name="scrp", bufs=2)
        exp_pool = tc.alloc_tile_pool(name="expp", bufs=2)

        dp8s = [dp_pool.tile([P, 2, D], F8, tag="dp", name=f"dp{i}")
                for i in range(NPAIR)]

        def s_mms(ej):
            s_ps = s_ps_pool.tile([P, D], F32, tag="s", name="s_ps")
            for pr in range(A_T // 2):
                lhs = k8s[pr][:, :, ej * P:(ej + 1) * P]
                for c in range(D // NCH):
                    nc.tensor.matmul(
                        s_ps[:, c * NCH:(c + 1) * NCH],
                        lhs,
                        q8s[pr][:, :, c * NCH:(c + 1) * NCH],
                        start=(pr == 0),
                        stop=(pr == A_T // 2 - 1),
                        perf_mode=DR,
                    )
            return s_ps

        def s_evict(ej, s_ps):
            s_scr = scr_pool.tile([P, D], F32, tag="s_scr", name="s_scr")
            nc.vector.tensor_tensor(s_scr[:], s_ps[:], bcast_sb[:], MULT)
            e_sb = exp_pool.tile([P, D], F32, tag="exp", name="e_sb")
            nc.scalar.activation(e_sb[:], s_scr[:], AF.Exp,
                                 scale=invnk_col[:, ej:ej + 1])
            nc.vector.tensor_scalar(
                out=dp8s[ej // 2][:, ej % 2, :], in0=e_sb[:],
                scalar1=1.0, scalar2=DP_SCALE, op0=SUB, op1=MULT,
            )

        # scores ej=0 run first so the PE is busy while the invnk chain
        # (sqrt -> transposes -> reciprocal) completes
        s_ps0 = s_mms(0)
        nkc_ps = smallcol_ps.tile([P, D // P], F32, tag="smc", name="nkc_ps")
        for j in range(D // P):
            nc.tensor.transpose(nkc_ps[:, j:j + 1],
                                nk_row[0:1, j * P:(j + 1) * P], one11[:])
        nc.vector.reciprocal(invnk_col[:], nkc_ps[:])
        s_evict(0, s_ps0)
        for ej in range(1, D // P):
            s_evict(ej, s_mms(ej))

        s_ps_pool.release()

        # ---------- phase 3: v projection (+ denom / colsum / iw chains) --
        v_ps_pool = tc.alloc_tile_pool(name="v_ps", bufs=2, space="PSUM")
        v8s = [v8_pool.tile([P, 2, A], F8, tag="v8", name=f"v8_{i}")
               for i in range(NPAIR)]
        dn_row = misc.tile([1, D], F32, tag="dn_row")
        cs_row = misc.tile([1, A], BF, tag="cs_row")
        iw_row = misc.tile([1, D], BF, tag="iw_row")

        def v_mms(dj, h):
            vp = v_ps_pool.tile([P, A // 2], F32, tag="vps", name="vp")
            for pr in range(NPAIR):
                lhs = wv8s[pr][:, :, dj * P:(dj + 1) * P]
                for c in range(2):
                    off = h * (A // 2) + c * NCH
                    nc.tensor.matmul(
                        vp[:, c * NCH:(c + 1) * NCH],
                        lhs,
                        x8s[pr][:, :, off:off + NCH],
                        start=(pr == 0),
                        stop=(pr == NPAIR - 1),
                        perf_mode=DR,
                    )
            nc.scalar.activation(
                v8s[dj // 2][:, dj % 2, h * (A // 2):(h + 1) * (A // 2)],
                vp[:], AF.Copy)

        def dn_mms():
            # denom row from quantized dP: ones8 @ dP  (+1024 after /32)
            dn_ps = nrm_ps.tile([1, D], F32, tag="nrm", name="dn_ps")
            for pr in range(NPAIR):
                for c in range(D // NCH):
                    nc.tensor.matmul(
                        dn_ps[:, c * NCH:(c + 1) * NCH],
                        ones8,
                        dp8s[pr][:, :, c * NCH:(c + 1) * NCH],
                        start=(pr == 0),
                        stop=(pr == NPAIR - 1),
                        perf_mode=DR,
                    )
            nc.vector.tensor_scalar(
                out=dn_row[:], in0=dn_ps[:],
                scalar1=1.0 / DP_SCALE, scalar2=float(D),
                op0=MULT, op1=ADD,
            )

        def dn_cols():
            dnc_ps = smallcol_ps.tile([P, D // P], F32, tag="smc",
                                      name="dnc_ps")
            for j in range(D // P):
                nc.tensor.transpose(dnc_ps[:, j:j + 1],
                                    dn_row[0:1, j * P:(j + 1) * P], one11[:])
            nc.vector.reciprocal(invden_col[:], dnc_ps[:])
            nc.vector.tensor_copy(invden_bf[:], invden_col[:])
            nc.vector.tensor_scalar(
                out=sc2_col[:], in0=invden_col[:],
                scalar1=D2_SCALE / (DP_SCALE * ALPHA), scalar2=None, op0=MULT,
            )

        def cs_mms(h):
            cs_ps = nrm_ps.tile([1, A // 2], F32, tag="nrm", name="cs_ps")
            for ft in range(8):
                lhs = wvc[:, ft:ft + 1]
                for c in range(2):
                    off = h * (A // 2) + c * NCH
                    nc.tensor.matmul(
                        cs_ps[:, c * NCH:(c + 1) * NCH],
                        lhs,
                        xbs[ft][:, off:off + NCH],
                        start=(ft == 0),
                        stop=(ft == 7),
                    )
            nc.vector.tensor_copy(
                cs_row[0:1, h * (A // 2):(h + 1) * (A // 2)], cs_ps[:])

        def iw_mms():
            iw_ps = nrm_ps.tile([1, D], F32, tag="nrm", name="iw_ps")
            for dj in range(8):
                lhs = invden_bf[:, dj:dj + 1]
                for c in range(D // NCH):
                    nc.tensor.matmul(
                        iw_ps[:, c * NCH:(c + 1) * NCH],
                        lhs,
                        wobs[dj][:, c * NCH:(c + 1) * NCH],
                        start=(dj == 0),
                        stop=(dj == 7),
                    )
            nc.vector.tensor_scalar(
                out=iw_row[:], in0=iw_ps[:],
                scalar1=D2_SCALE * ALPHA, scalar2=None, op0=MULT,
            )

        interludes = {1: dn_mms, 3: dn_cols, 5: lambda: cs_mms(0),
                      7: lambda: cs_mms(1), 9: iw_mms}
        step = 0
        for dj in range(8):
            for h in range(2):
                v_mms(dj, h)
                step += 1
                if step in interludes:
                    interludes[step]()

        v_ps_pool.release()
        smallcol_ps.release()
        nrm_ps.release()
        exp_pool.release()
        scr_pool.release()
        bcast_pool.release()
        qk_pool.release()

        # ---------- phase 4: delta2 = invden * (dP @ v) ----------
        d2_ps_pool = tc.alloc_tile_pool(name="d2_ps", bufs=2, space="PSUM")
        d2s = [d2_pool.tile([P, 2, A], F8, tag="d2", name=f"d2_{i}")
               for i in range(NPAIR)]
        for dj in range(8):
            dp_ = d2_ps_pool.tile([P, A], F32, tag="d2ps", name="dp_")
            for pr in range(NPAIR):
                lhs = dp8s[pr][:, :, dj * P:(dj + 1) * P]
                for c in range(A // NCH):
                    nc.tensor.matmul(
                        dp_[:, c * NCH:(c + 1) * NCH],
                        lhs,
                        v8s[pr][:, :, c * NCH:(c + 1) * NCH],
                        start=(pr == 0),
                        stop=(pr == NPAIR - 1),
                        perf_mode=DR,
                    )
            nc.scalar.activation(d2s[dj // 2][:, dj % 2, :], dp_[:], AF.Copy,
                                 scale=sc2_col[:, dj:dj + 1])
        d2_ps_pool.release()

        # ---------- phase 5: yT = wo8.T-ish GEMM + rank-1 ----------
        y_ps_pool = tc.alloc_tile_pool(name="y_ps", bufs=2, space="PSUM")
        y_pool = tc.alloc_tile_pool(name="yp", bufs=2)
        for fj in range(8):
            yp = y_ps_pool.tile([P, A], F32, tag="yps", name="yp_t")
            for pr in range(NPAIR):
                lhs = wo8s[pr][:, :, fj * P:(fj + 1) * P]
                for c in range(A // NCH):
                    nc.tensor.matmul(
                        yp[:, c * NCH:(c + 1) * NCH],
                        lhs,
                        d2s[pr][:, :, c * NCH:(c + 1) * NCH],
                        start=(pr == 0),
                        stop=False,
                        perf_mode=DR,
                    )
            for c in range(A // NCH):
                nc.tensor.matmul(
                    yp[:, c * NCH:(c + 1) * NCH],
                    iw_row[0:1, fj * P:(fj + 1) * P],
                    cs_row[0:1, c * NCH:(c + 1) * NCH],
                    start=False,
                    stop=True,
                )
            y_sb = y_pool.tile([P, A], BF, tag="y", name="y_sb")
            for h in range(2):
                sl = slice(h * (A // 2), (h + 1) * (A // 2))
                if fj % 2 == 0:
                    nc.vector.tensor_scalar(
                        out=y_sb[:, sl], in0=yp[:, sl],
                        scalar1=1.0 / (D2_SCALE * ALPHA), scalar2=None,
                        op0=MULT,
                    )
                else:
                    nc.scalar.activation(y_sb[:, sl], yp[:, sl], AF.Copy,
                                         scale=1.0 / (D2_SCALE * ALPHA))
                nc.sync.dma_start(
                    y_d[fj * P:(fj + 1) * P, sl], y_sb[:, sl])

        y_pool.release()
        y_ps_pool.release()
        wob_pool.release()
        wv8_pool.release()
        xb_pool.release()
        x8_pool.release()
        dp_pool.release()
        v8_pool.release()
        d2_pool.release()
        wo8_pool.release()
        misc.release()
        consts.release()

    nc.compile()
    return nc


def _pair_layout(mT):
    """[K, M] -> DoubleRow pair layout [K/256*128, 2*M] (row pr*128+p)."""
    K, M = mT.shape
    return np.ascontiguousarray(
        mT.reshape(K // 256, 2, P, M).transpose(0, 2, 1, 3).reshape(K // 2, 2 * M))


def _host_inputs(x, Wq, Wk, Wv, Wo, temperature):
    import ml_dtypes
    f8 = ml_dtypes.float8_e4m3
    bf16 = ml_dtypes.bfloat16

    def to8(a):
        return np.clip(a, -239.0, 239.0).astype(f8)

    wq8 = _pair_layout(to8(ALPHA * np.asarray(Wq).T))
    wk8 = _pair_layout(to8(ALPHA * np.asarray(Wk).T))
    wv8 = _pair_layout(to8(ALPHA * np.asarray(Wv).T))
    wo8 = _pair_layout(to8(ALPHA * np.asarray(Wo).T))
    wob = np.ascontiguousarray(np.asarray(Wo).T).astype(bf16)
    wvc = np.ascontiguousarray(
        np.asarray(Wv).sum(0).reshape(D // P, P).T).astype(bf16)
    in_maps = []
    for b in range(B):
        xT = np.ascontiguousarray(np.asarray(x[b]).T)
        in_maps.append({
            "x8": _pair_layout(to8(xT)),
            "xb": xT.astype(bf16),
            "wq8": wq8, "wk8": wk8, "wv8": wv8, "wo8": wo8,
            "wob": wob, "wvc": wvc,
            "temp": np.asarray(temperature[b]).reshape(1, 1).astype(np.float32),
        })
    return in_maps


def run(x, Wq, Wk, Wv, Wo, temperature, trace=False, tmpdir=None):
    _ensure_path()
    from concourse.bass_utils import run_bass_kernel_spmd

    if "nc" not in _CACHE:
        _CACHE["nc"] = build_bass()
    nc = _CACHE["nc"]
    in_maps = _host_inputs(x, Wq, Wk, Wv, Wo, temperature)
    res = run_bass_kernel_spmd(
        nc, in_maps, core_ids=list(range(B)), trace=trace, tmpdir=tmpdir
    )
    out = np.stack([
        np.asarray(res.results[b]["y"]).astype(np.float32).T for b in range(B)
    ])
    return out, res


def kernel(x, Wq, Wk, Wv, Wo, temperature):
    out, _ = run(x, Wq, Wk, Wv, Wo, temperature, trace=False)
    return out
